# revision 1
# baseline (speedup 1.0000x reference)
"""v2 feature kernel: dual-player packed planes (my: bits 0-7, op: bits 16-23)
and column-oriented line features computed in the row-byte layout via
row-offset access patterns. Halves the boolean-logic op count of v1 and
removes the column packing + merge expansion passes.

Plane geometry: padded tiles [P, NB, 18] u32, valid rows 5:13, 5 guard rows
of zeros on each side (column windows reach +-5 rows).
"""
import numpy as np

import concourse.bass as bass
import concourse.bacc as bacc
import concourse.mybir as mybir
import concourse.tile as tile

Alu = mybir.AluOpType
Act = mybir.ActivationFunctionType
DT = mybir.dt

P = 128
NB = 32
CB = 4
NCHUNK = NB // CB
NCORES = 8
BPC = P * NB
PAD = 18
R0 = 5
ROWS = slice(R0, R0 + 8)
SEG = 0x00FF00FF  # both player segments

DIRS = ((0, 1), (1, 0), (1, 1), (1, -1))


def _build_masks() -> np.ndarray:
    """[P, 2*64] u32: lane masks for my (1<<j) and op (1<<(16+j))."""
    j = np.tile(np.arange(8), 8)
    t = np.concatenate([(1 << j), (1 << (16 + j))]).reshape(1, 128).astype(np.uint32)
    return np.broadcast_to(t, (P, 128)).copy()


def _stt_raw(eng, out, in0, imm, in1, op0, op1, imm_dt=DT.uint32):
    outs = [eng.lower_ap(out)]
    return eng.add_instruction(
        mybir.InstTensorScalarPtr(
            name=eng.bass.get_next_instruction_name(),
            is_scalar_tensor_tensor=True,
            op0=op0, op1=op1,
            ins=[eng.lower_ap(in0),
                 mybir.ImmediateValue(dtype=imm_dt, value=imm),
                 eng.lower_ap(in1)],
            outs=outs,
        )
    )


def _stt(eng, out, in0, sh, op1, in1):
    if sh > 0:
        _stt_raw(eng, out, in0, sh, in1, Alu.logical_shift_left, op1)
    elif sh < 0:
        _stt_raw(eng, out, in0, -sh, in1, Alu.logical_shift_right, op1)
    else:
        eng.tensor_tensor(out, in0, in1, op1)


def feature_kernel(tc, out_d, state_d, side_d):
    nc = tc.nc
    V, G, A = nc.vector, nc.gpsimd, nc.scalar

    state_v = state_d.rearrange("(p n) c -> p n c", p=P)
    side_v = side_d.rearrange("(p n) -> p n", p=P)
    out_v = out_d.rearrange("(p n) c -> p n c", p=P)

    with (
        tc.tile_pool(name="main", bufs=1) as pool,
        tc.tile_pool(name="chk", bufs=2) as cpool,
    ):
        # ---------- input ----------
        sideT = pool.tile([P, NB], DT.float32, name="sideT")
        nc.sync.dma_start(sideT[:], side_v)
        # expansion bit masks built on-device: 1<<j per cell, and <<16 for op
        jv = pool.tile([P, 64], DT.uint32, name="jv")
        G.iota(jv[:], pattern=[[0, 8], [1, 8]], base=0, channel_multiplier=0)
        onesp = pool.tile([P, 64], DT.uint32, name="onesp")
        V.memset(onesp[:], 1)
        masks = pool.tile([P, 128], DT.uint32, name="masks")
        V.tensor_tensor(masks[:, 0:64], onesp[:], jv[:], Alu.logical_shift_left)
        V.tensor_scalar(masks[:, 64:128], masks[:, 0:64], 16, None,
                        Alu.logical_shift_left)

        s = pool.tile([P, NB, 64], DT.float32, name="s")
        nc.sync.dma_start(s[:], state_v)
        negside = pool.tile([P, NB], DT.float32, name="negside")
        V.tensor_scalar(negside[:], sideT[:], -1.0, None, Alu.mult)
        myf = pool.tile([P, NB, 64], DT.float32, name="myf")
        opf = pool.tile([P, NB, 64], DT.float32, name="opf")
        V.tensor_tensor(
            myf[:], s[:], sideT[:, :, None].broadcast_to((P, NB, 64)), Alu.is_equal
        )
        V.tensor_tensor(
            opf[:], s[:], negside[:, :, None].broadcast_to((P, NB, 64)), Alu.is_equal
        )

        # ---------- padded plane allocator ----------
        def ptile(name, lo=1, hi=5):
            """Padded tile; zero only guard rows [R0-lo, R0) and [13, 13+hi)."""
            t = pool.tile([P, NB, PAD], DT.uint32, name=name)
            V.memset(t[:, :, R0 - lo:R0], 0)
            V.memset(t[:, :, R0 + 8:R0 + 8 + hi], 0)
            return t

        # ---------- packing ----------
        myR = pool.tile([P, NB, 8], DT.uint32, name="myR")
        opR = pool.tile([P, NB, 8], DT.uint32, name="opR")

        def pack(dst_ap, srcf):
            v = srcf.rearrange("p n (r j2 t) -> p (n r) j2 t", t=2, j2=4)
            a1, b1 = v[:, :, :, 1], v[:, :, :, 0]
            t1 = pool.tile([P, NB * 8, 4], DT.float32, name="pk_t1")
            V.scalar_tensor_tensor(t1[:], a1, 2.0, b1, op0=Alu.mult, op1=Alu.add)
            w2 = t1.rearrange("p q (k t) -> p q k t", t=2)
            a2, b2 = w2[:, :, :, 1], w2[:, :, :, 0]
            t2 = pool.tile([P, NB * 8, 2], DT.float32, name="pk_t2")
            V.scalar_tensor_tensor(t2[:], a2, 4.0, b2, op0=Alu.mult, op1=Alu.add)
            w3 = t2.rearrange("p (n r) t -> p n r t", r=8)
            a3, b3 = w3[:, :, :, 1], w3[:, :, :, 0]
            V.scalar_tensor_tensor(dst_ap, a3, 16.0, b3, op0=Alu.mult, op1=Alu.add)

        pack(myR[:], myf)
        pack(opR[:], opf)

        # dual planes: Ad = my | op<<16 ; Bd = op | my<<16 (op from my-persp etc.)
        Ad = ptile("Ad")
        Bd = ptile("Bd")
        _stt(V, Ad[:, :, ROWS], opR[:], 16, Alu.bitwise_or, myR[:])
        _stt(V, Bd[:, :, ROWS], myR[:], 16, Alu.bitwise_or, opR[:])
        Ed = ptile("Ed")      # empty (same in both segments)
        V.tensor_tensor(Ed[:, :, ROWS], Ad[:, :, ROWS], Bd[:, :, ROWS], Alu.bitwise_or)
        V.tensor_scalar(Ed[:, :, ROWS], Ed[:, :, ROWS], SEG, None, Alu.bitwise_xor)
        Nd = ptile("Nd")      # notme of Ad
        V.tensor_scalar(Nd[:, :, ROWS], Ad[:, :, ROWS], SEG, None, Alu.bitwise_xor)

        # channel plane group (dual): [c1 c2 c3 l2 l3 r3]
        Rg = pool.tile([P, 6, NB, 8], DT.uint32, name="Rg")

        # ---------- connectivity (dual, 4 dirs) ----------
        d2 = ptile("cn_d2", 1, 2); d3 = ptile("cn_d3", 1, 2); d4 = ptile("cn_d4", 1, 2)
        t3 = ptile("cn_t3", 1, 2); t4 = ptile("cn_t4", 1, 2)
        Atiles = {}
        for di_i in range(4):
            for N in (2, 3, 4):
                Atiles[(di_i, N)] = pool.tile([P, NB, 8], DT.uint32,
                                              name=f"cn_a{N}_{di_i}")

        def AV(di_i, N):
            return Atiles[(di_i, N)][:]

        cx1 = pool.tile([P, NB, 8], DT.uint32, name="cx1")
        cx2 = pool.tile([P, NB, 8], DT.uint32, name="cx2")
        cx3 = pool.tile([P, NB, 8], DT.uint32, name="cx3")

        mv = Ad[:, :, ROWS]
        for di_i, (di, dj) in enumerate(DIRS):
            def fwd(t):
                return t[:, :, R0 - di:R0 + 8 - di]

            def bwd(t, k=1):
                return t[:, :, R0 + k * di:R0 + 8 + k * di]

            a2, a3, a4 = (AV(di_i, N) for N in (2, 3, 4))
            _stt(V, d2[:, :, ROWS], fwd(Ad), dj, Alu.bitwise_and, mv)
            _stt(V, d3[:, :, ROWS], fwd(d2), dj, Alu.bitwise_and, d2[:, :, ROWS])
            _stt(V, d4[:, :, ROWS], fwd(d3), dj, Alu.bitwise_and, d3[:, :, ROWS])
            _stt(V, a2, bwd(d2), -dj, Alu.bitwise_or, d2[:, :, ROWS])
            _stt(V, t3[:, :, ROWS], bwd(d3), -dj, Alu.bitwise_or, d3[:, :, ROWS])
            _stt(V, a3, bwd(d3, 2), -2 * dj, Alu.bitwise_or, t3[:, :, ROWS])
            _stt(V, t4[:, :, ROWS], bwd(d4), -dj, Alu.bitwise_or, d4[:, :, ROWS])
            _stt(V, a4, bwd(t4, 2), -2 * dj, Alu.bitwise_or, t4[:, :, ROWS])

        V.tensor_tensor(cx1[:], AV(0, 2), AV(1, 2), Alu.bitwise_and)
        V.tensor_tensor(cx1[:], cx1[:], AV(2, 2), Alu.bitwise_and)
        V.tensor_tensor(cx1[:], cx1[:], AV(3, 2), Alu.bitwise_and)
        V.tensor_tensor(Rg[:, 0], mv, cx1[:], Alu.bitwise_xor)
        for k, N in ((1, 2), (2, 3)):
            V.tensor_tensor(cx1[:], AV(0, N), AV(0, N + 1), Alu.bitwise_xor)
            V.tensor_tensor(cx2[:], AV(1, N), AV(1, N + 1), Alu.bitwise_xor)
            V.tensor_tensor(cx1[:], cx1[:], cx2[:], Alu.bitwise_or)
            V.tensor_tensor(cx2[:], AV(2, N), AV(2, N + 1), Alu.bitwise_xor)
            V.tensor_tensor(cx3[:], AV(3, N), AV(3, N + 1), Alu.bitwise_xor)
            V.tensor_tensor(cx2[:], cx2[:], cx3[:], Alu.bitwise_or)
            V.tensor_tensor(Rg[:, k], cx1[:], cx2[:], Alu.bitwise_or)

        # ---------- line features ----------
        # padded tmp tiles (shared by row/col calls; guards stay zero)
        TMP = {}
        R0T = 2

        def tp(name):
            if name not in TMP:
                t = pool.tile([P, NB, 12], DT.uint32, name="lf_" + name)
                blocks = t.rearrange("p n (a b) -> p n a b", a=6, b=2)
                V.memset(blocks[:, :, 0::5, :], 0)   # rows 0:2 and 10:12
                TMP[name] = t
            return TMP[name]

        lf_row = {}  # row-mode results: l2 (final bits), l3 (bits unshifted), r3

        def line_feats_row():
            me, op, em, nm = (x[:, :, ROWS] for x in (Ad, Bd, Ed, Nd))

            def T(n):
                return tp(n)[:, :, R0T:R0T + 8]

            _stt(V, T("t"), me, -1, Alu.bitwise_and, me)
            _stt(V, T("u"), em, -1, Alu.bitwise_and, em)
            _stt(V, T("a"), T("u"), -2, Alu.bitwise_and, T("t"))
            _stt(V, T("w"), em, -3, Alu.bitwise_and, em)
            _stt(V, T("b"), T("t"), -1, Alu.bitwise_and, T("w"))
            _stt(V, T("y"), T("b"), 1, Alu.bitwise_or, T("b"))
            V.tensor_tensor(T("q"), T("a"), T("y"), Alu.bitwise_or)
            _stt(V, T("l2"), T("q"), 1, Alu.bitwise_or, T("a"))

            _stt(V, T("m3"), me, -2, Alu.bitwise_and, T("t"))
            _stt(V, T("r1"), em, -4, Alu.bitwise_and, em)
            _stt(V, T("c"), T("m3"), -1, Alu.bitwise_and, T("r1"))
            _stt(V, T("i1"), T("c"), 1, Alu.bitwise_or, T("c"))
            _stt(V, T("l3"), T("i1"), 1, Alu.bitwise_or, T("c"))  # unshifted

            V.tensor_scalar(T("lb"), op, 1, 0x00010001,
                            op0=Alu.logical_shift_left, op1=Alu.bitwise_or)
            _stt(V, T("d0"), em, -3, Alu.bitwise_and, T("m3"))
            _stt(V, T("d1"), nm, -4, Alu.bitwise_and, T("d0"))
            V.tensor_tensor(T("d"), T("d1"), T("lb"), Alu.bitwise_and)
            _stt(V, T("j1"), T("d"), 1, Alu.bitwise_or, T("d"))
            _stt(V, T("md"), T("d"), 2, Alu.bitwise_or, T("j1"))
            _stt(V, T("o3"), T("m3"), -1, Alu.bitwise_and, nm)
            _stt(V, T("o3"), nm, -4, Alu.bitwise_and, T("o3"))
            V.tensor_scalar(T("rb"), op, 5, 0x00F800F8,
                            op0=Alu.logical_shift_right, op1=Alu.bitwise_or)
            V.tensor_tensor(T("x"), T("lb"), T("rb"), Alu.bitwise_xor)
            V.tensor_tensor(T("e"), T("o3"), T("x"), Alu.bitwise_and)
            _stt(V, T("g1"), T("e"), 1, Alu.bitwise_or, T("e"))
            _stt(V, T("g2"), T("g1"), 1, Alu.bitwise_or, T("e"))
            _stt(V, T("r3"), T("g2"), 1, Alu.bitwise_or, T("md"))
            lf_row["l2"] = tp("l2"); lf_row["l3"] = tp("l3"); lf_row["r3"] = tp("r3")

        line_feats_row()

        # column mode: positions along rows; shifts become row-offset views.
        # up_k(x): value from k rows earlier (toward row 0); dn_k: k rows later.
        lbmC = pool.tile([P, 8], DT.uint32, name="lbmC")
        rbmC = pool.tile([P, 8], DT.uint32, name="rbmC")
        V.memset(lbmC[:], 0)
        V.memset(lbmC[:, 0:1], SEG)
        V.memset(rbmC[:], 0)
        V.memset(rbmC[:, 3:8], SEG)

        def line_feats_col():
            def dn(x, k):  # x[r+k]
                return x[:, :, R0 + k:R0 + 8 + k]

            def T(n, k=0):
                nm = "c_" + n if n in ("l2", "l3", "r3") else n
                t = tp(nm)
                return t[:, :, R0T + k:R0T + 8 + k]

            me, op, em, nm = Ad, Bd, Ed, Nd

            def MV(x, k=0):
                return x[:, :, R0 + k:R0 + 8 + k]

            V.tensor_tensor(T("t"), MV(me), dn(me, 1), Alu.bitwise_and)
            V.tensor_tensor(T("u"), MV(em), dn(em, 1), Alu.bitwise_and)
            V.tensor_tensor(T("a"), T("t"), T("u", 2), Alu.bitwise_and)
            V.tensor_tensor(T("w"), MV(em), dn(em, 3), Alu.bitwise_and)
            V.tensor_tensor(T("b"), T("w"), T("t", 1), Alu.bitwise_and)
            V.tensor_tensor(T("y"), T("b"), T("b", -1), Alu.bitwise_or)
            V.tensor_tensor(T("q"), T("a"), T("y"), Alu.bitwise_or)
            V.tensor_tensor(T("l2"), T("a"), T("q", -1), Alu.bitwise_or)

            V.tensor_tensor(T("m3"), T("t"), dn(me, 2), Alu.bitwise_and)
            V.tensor_tensor(T("r1"), MV(em), dn(em, 4), Alu.bitwise_and)
            V.tensor_tensor(T("c"), T("r1"), T("m3", 1), Alu.bitwise_and)
            V.tensor_tensor(T("i1"), T("c"), T("c", -1), Alu.bitwise_or)
            V.tensor_tensor(T("l3"), T("c"), T("i1", -1), Alu.bitwise_or)  # row-unshifted

            V.tensor_tensor(
                T("lb"), MV(op, -1),
                lbmC[:, None, :].broadcast_to((P, NB, 8)), Alu.bitwise_or,
            )
            V.tensor_tensor(T("d0"), T("m3"), dn(em, 3), Alu.bitwise_and)
            V.tensor_tensor(T("d1"), T("d0"), dn(nm, 4), Alu.bitwise_and)
            V.tensor_tensor(T("d"), T("d1"), T("lb"), Alu.bitwise_and)
            V.tensor_tensor(T("j1"), T("d"), T("d", -1), Alu.bitwise_or)
            V.tensor_tensor(T("md"), T("j1"), T("d", -2), Alu.bitwise_or)
            V.tensor_tensor(T("o3"), T("m3", 1), MV(nm), Alu.bitwise_and)
            V.tensor_tensor(T("o3"), T("o3"), dn(nm, 4), Alu.bitwise_and)
            V.tensor_tensor(
                T("rb"), MV(op, 5),
                rbmC[:, None, :].broadcast_to((P, NB, 8)), Alu.bitwise_or,
            )
            V.tensor_tensor(T("x"), T("lb"), T("rb"), Alu.bitwise_xor)
            V.tensor_tensor(T("e"), T("o3"), T("x"), Alu.bitwise_and)
            V.tensor_tensor(T("g1"), T("e"), T("e", -1), Alu.bitwise_or)
            V.tensor_tensor(T("g2"), T("e"), T("g1", -1), Alu.bitwise_or)
            V.tensor_tensor(T("r3"), T("md"), T("g2", -1), Alu.bitwise_or)

        line_feats_col()

        # merges into Rg lanes 3..5
        ctp = TMP  # col tiles are "c_*"
        RT = slice(2, 10)
        V.tensor_tensor(Rg[:, 3], lf_row["l2"][:, :, RT],
                        ctp["c_l2"][:, :, RT], Alu.bitwise_or)
        _stt(V, Rg[:, 4], lf_row["l3"][:, :, RT], 1, Alu.bitwise_or,
             ctp["c_l3"][:, :, 1:9])
        V.tensor_tensor(Rg[:, 5], lf_row["r3"][:, :, RT],
                        ctp["c_r3"][:, :, RT], Alu.bitwise_or)

        # ---------- expansion ----------
        OUTCH = {0: (2, 5), 1: (8, 11)}       # persp 0: ch2-4 conn / 8-10 line
        OUTCH_OP = {0: (5, 8), 1: (13, 16)}   # persp 1: ch5-7 / 13-15

        for ck in range(NCHUNK):
            n0 = ck * CB
            outt = cpool.tile([P, CB, 18, 64], DT.float32, name="outt")
            mk = cpool.tile([P, 2, 6, CB, 64], DT.uint32, name="mk", bufs=2)
            dsums = cpool.tile([P, CB, 6], DT.float32, name="dsums", bufs=2)
            dge = cpool.tile([P, CB, 4], DT.float32, name="dge", bufs=2)

            A.activation(outt[:, :, 0, :], myf[:, n0:n0 + CB, :], Act.Copy)
            A.activation(outt[:, :, 1, :], opf[:, n0:n0 + CB, :], Act.Copy)

            for pi in range(2):
                V.tensor_tensor(
                    mk[:, pi].rearrange("p c b (r j) -> p c b r j", j=8),
                    Rg[:, :, n0:n0 + CB, :, None].broadcast_to((P, 6, CB, 8, 8)),
                    masks[:, pi * 64:(pi + 1) * 64]
                    .rearrange("p (r j) -> p r j", j=8)[:, None, None]
                    .broadcast_to((P, 6, CB, 8, 8)),
                    Alu.bitwise_and,
                )
            # conn channels
            V.tensor_scalar(
                outt[:, :, 2:5, :], mk[:, 0, 0:3].rearrange("p c b x -> p b c x"),
                0, None, Alu.not_equal)
            V.tensor_scalar(
                outt[:, :, 5:8, :], mk[:, 1, 0:3].rearrange("p c b x -> p b c x"),
                0, None, Alu.not_equal)
            # line channels
            V.tensor_scalar(
                outt[:, :, 8:11, :], mk[:, 0, 3:6].rearrange("p c b x -> p b c x"),
                0, None, Alu.not_equal)
            V.tensor_scalar(
                outt[:, :, 13:16, :], mk[:, 1, 3:6].rearrange("p c b x -> p b c x"),
                0, None, Alu.not_equal)
            # doubles: per-board cell sums of the merged channels
            V.tensor_reduce(
                dsums[:, :, 0:3], outt[:, :, 8:11, :],
                axis=mybir.AxisListType.X, op=Alu.add)
            V.tensor_reduce(
                dsums[:, :, 3:6], outt[:, :, 13:16, :],
                axis=mybir.AxisListType.X, op=Alu.add)
            s23m = dge[:, :, 1]
            s23o = dge[:, :, 3]
            V.tensor_tensor(s23m, dsums[:, :, 1], dsums[:, :, 2], Alu.add)
            V.tensor_tensor(s23o, dsums[:, :, 4], dsums[:, :, 5], Alu.add)
            V.tensor_scalar(dge[:, :, 0], dsums[:, :, 0], 1.5, None, Alu.is_ge)
            V.tensor_scalar(dge[:, :, 1], s23m, 1.5, None, Alu.is_ge)
            V.tensor_scalar(dge[:, :, 2], dsums[:, :, 3], 1.5, None, Alu.is_ge)
            V.tensor_scalar(dge[:, :, 3], s23o, 1.5, None, Alu.is_ge)
            A.activation(
                outt[:, :, 11:13, :],
                dge[:, :, 0:2, None].broadcast_to((P, CB, 2, 64)), Act.Copy)
            A.activation(
                outt[:, :, 16:18, :],
                dge[:, :, 2:4, None].broadcast_to((P, CB, 2, 64)), Act.Copy)
            nc.sync.dma_start(
                out_v[:, n0:n0 + CB, :], outt.rearrange("p b c x -> p b (c x)"))


_NC_CACHE = None


def _build_nc():
    global _NC_CACHE
    if _NC_CACHE is not None:
        return _NC_CACHE
    nc = bacc.Bacc("TRN2", debug=False, enable_asserts=False)
    state_d = nc.dram_tensor("state", [BPC, 64], DT.float32, kind="ExternalInput").ap()
    side_d = nc.dram_tensor("side", [BPC], DT.float32, kind="ExternalInput").ap()
    out_d = nc.dram_tensor("out", [BPC, 18 * 64], DT.float32, kind="ExternalOutput").ap()
    with tile.TileContext(nc) as tc:
        feature_kernel(tc, out_d, state_d, side_d)
    nc.finalize()
    _NC_CACHE = nc
    return nc


_JIT_CACHE = None


def _get_runner():
    """Build a jitted shard_map runner over the 8 cores, fed with
    pre-sharded jax Arrays (avoids XLA-side resharding programs, which the
    neuron compiler chokes on for these sizes)."""
    global _JIT_CACHE
    if _JIT_CACHE is not None:
        return _JIT_CACHE
    import jax
    from jax.sharding import Mesh, PartitionSpec, NamedSharding
    try:
        from jax.experimental.shard_map import shard_map
    except ImportError:
        from jax.shard_map import shard_map  # newer jax
    from concourse import bass2jax as B2J

    B2J.install_neuronx_cc_hook()
    nc = _build_nc()

    in_names = ["state", "side"]
    out_names = ["out"]
    out_avals = [jax.core.ShapedArray((BPC, 18 * 64), np.float32)]
    all_names = in_names + out_names
    if nc.partition_id_tensor is not None:
        all_names = all_names + [nc.partition_id_tensor.name]

    def _body(state_a, side_a, zeros_a):
        operands = [state_a, side_a, zeros_a]
        if nc.partition_id_tensor is not None:
            operands.append(B2J.partition_id_tensor())
        outs = B2J._bass_exec_p.bind(
            *operands,
            out_avals=tuple(out_avals),
            in_names=tuple(all_names),
            out_names=tuple(out_names),
            lowering_input_output_aliases=(),
            sim_require_finite=True,
            sim_require_nnan=True,
            nc=nc,
        )
        return outs[0]

    devices = jax.devices()[:NCORES]
    mesh = Mesh(np.asarray(devices), ("core",))
    spec = PartitionSpec("core")
    sharded = jax.jit(
        shard_map(
            _body, mesh=mesh,
            in_specs=(spec, spec, spec),
            out_specs=spec,
            check_rep=False,
        ),
        donate_argnums=(2,),
        keep_unused=True,
    )

    def put(shards):
        arrs = [jax.device_put(s, devices[i]) for i, s in enumerate(shards)]
        global_shape = (sum(s.shape[0] for s in shards),) + shards[0].shape[1:]
        return jax.make_array_from_single_device_arrays(
            global_shape, NamedSharding(mesh, spec), arrs
        )

    _JIT_CACHE = (sharded, put)
    return _JIT_CACHE


def kernel(state, side):
    """Full-input entry point: state [32768,8,8] f32, side [32768] f32."""
    state = np.ascontiguousarray(np.asarray(state, dtype=np.float32)).reshape(-1, 64)
    side = np.ascontiguousarray(np.asarray(side, dtype=np.float32)).reshape(-1)
    B = state.shape[0]
    assert B == BPC * NCORES, (B, BPC * NCORES)
    sharded, put = _get_runner()
    state_g = put([state[i * BPC:(i + 1) * BPC] for i in range(NCORES)])
    side_g = put([side[i * BPC:(i + 1) * BPC] for i in range(NCORES)])
    zeros_g = put([np.zeros((BPC, 18 * 64), np.float32) for _ in range(NCORES)])
    out = sharded(state_g, side_g, zeros_g)
    out = np.asarray(out).reshape(NCORES * BPC, 18, 8, 8)
    return out

